# revision 18
# baseline (speedup 1.0000x reference)
"""MoE (B=4,S=2048,D=768,F=3072,E=8,K=2) on 8 TRN2 NeuronCores.

Strategy: data-parallel over tokens (1024 tokens/core), sparse top-2 expert
compute with capacity-padded per-expert buckets (C=384 slots/expert/core).
Router, top-2 selection, dispatch-index build, FFN, and weighted combine all
run on-device; the host only shards inputs, sums 8 tiny per-core partial
vectors for the aux losses, and reassembles the output.
"""

import numpy as np
import ml_dtypes

import concourse.bass as bass
import concourse.mybir as mybir
import concourse.tile as tile
from concourse import bacc
from concourse.bass_utils import run_bass_kernel_spmd

BF16 = ml_dtypes.bfloat16

B, S, D, F, E, K = 4, 2048, 768, 3072, 8, 2
NCORES = 8
T = (B * S) // NCORES          # 1024 tokens per core
NT = T // 128                  # 8 token tiles
C = 384                        # per-expert capacity (multiple of 128)
NSLOT = E * C                  # 3072 dispatch slots
DUMP = T                       # dump row index (row T of xbf / out)
KD = D // 128                  # 6 k-tiles over D
MF = F // 128                  # 24 m-tiles over F
CT = C // 128                  # 3 slot tiles per expert
BIGOFF = 16384.0               # scatter-skip offset (> bounds_check, f32-exact range)

DEBUG = False

f32 = mybir.dt.float32
bf16 = mybir.dt.bfloat16
i16 = mybir.dt.int16
i32 = mybir.dt.int32


def router_phase(nc, tc, xT, tokdram, cwdram, aux,
                 lt_sb, ones_sb, ecap_sb, tokid_sb, wr_sb, idxs_sb, cw_sb):
    with (
        tc.tile_pool(name="rbig", bufs=1) as rbig,
        tc.tile_pool(name="rpsum", bufs=2, space="PSUM") as rpsum,
        tc.tile_pool(name="rtmp", bufs=2) as rtmp,
    ):
        xT_sb = rbig.tile([128, KD, T], f32)
        nc.sync.dma_start(out=xT_sb[:], in_=xT[:, :].rearrange("(k p) t -> p k t", p=128))

        carry = rbig.tile([1, E], f32)
        nc.vector.memset(carry[:], 0.0)
        usage_acc = rbig.tile([128, E], f32)
        nc.vector.memset(usage_acc[:], 0.0)
        zsq_acc = rbig.tile([128, 1], f32)
        nc.vector.memset(zsq_acc[:], 0.0)
        offs_tok_f = rbig.tile([128, 2 * NT], f32)
        offs_cw_f = rbig.tile([128, 2 * NT], f32)
        vals_cw = rbig.tile([128, 2 * NT], f32)

        for c in range(NT):
            tok_sl = slice(128 * c, 128 * (c + 1))
            lg_ps = rpsum.tile([128, E], f32, tag="lg")
            for k in range(KD):
                nc.tensor.matmul(
                    lg_ps[:], lhsT=xT_sb[:, k, tok_sl], rhs=wr_sb[:, k, :],
                    start=(k == 0), stop=(k == KD - 1),
                )
            l = rtmp.tile([128, E], f32, tag="l")
            nc.scalar.copy(out=l[:], in_=lg_ps[:])

            m1 = rtmp.tile([128, 1], f32, tag="m1")
            nc.vector.reduce_max(m1[:], l[:], axis=mybir.AxisListType.X)
            is1 = rtmp.tile([128, E], f32, tag="is1")
            nc.vector.tensor_scalar(is1[:], l[:], m1[:], None, mybir.AluOpType.is_ge)
            # mask out the top-1 and find the second max
            lmask = rtmp.tile([128, E], f32, tag="lmask")
            nc.vector.scalar_tensor_tensor(
                out=lmask[:], in0=is1[:], scalar=-1e9, in1=l[:],
                op0=mybir.AluOpType.mult, op1=mybir.AluOpType.add,
            )
            m2 = rtmp.tile([128, 1], f32, tag="m2")
            nc.vector.reduce_max(m2[:], lmask[:], axis=mybir.AxisListType.X)

            # softmax (full 8) + logsumexp
            negm1 = rtmp.tile([128, 1], f32, tag="negm1")
            nc.vector.tensor_scalar_mul(negm1[:], m1[:], -1.0)
            ex = rtmp.tile([128, E], f32, tag="ex")
            sume = rtmp.tile([128, 1], f32, tag="sume")
            nc.scalar.activation(
                ex[:], l[:], mybir.ActivationFunctionType.Exp,
                bias=negm1[:], scale=1.0, accum_out=sume[:],
            )
            rcps = rtmp.tile([128, 1], f32, tag="rcps")
            nc.vector.reciprocal(rcps[:], sume[:])
            p = rtmp.tile([128, E], f32, tag="p")
            nc.vector.tensor_scalar_mul(p[:], ex[:], rcps[:])
            nc.vector.tensor_add(usage_acc[:], usage_acc[:], p[:])
            lns = rtmp.tile([128, 1], f32, tag="lns")
            nc.scalar.activation(lns[:], sume[:], mybir.ActivationFunctionType.Ln)
            z = rtmp.tile([128, 1], f32, tag="z")
            nc.vector.tensor_add(z[:], lns[:], m1[:])
            z2 = rtmp.tile([128, 1], f32, tag="z2")
            nc.vector.tensor_mul(z2[:], z[:], z[:])
            nc.vector.tensor_add(zsq_acc[:], zsq_acc[:], z2[:])

            # top-2 combine weights
            mask2 = rtmp.tile([128, E], f32, tag="mask2")
            nc.vector.tensor_scalar(mask2[:], l[:], m2[:], None, mybir.AluOpType.is_ge)
            psel = rtmp.tile([128, E], f32, tag="psel")
            nc.vector.tensor_mul(psel[:], p[:], mask2[:])
            s12 = rtmp.tile([128, 1], f32, tag="s12")
            nc.vector.reduce_sum(s12[:], psel[:], axis=mybir.AxisListType.X)
            nc.vector.tensor_scalar_add(s12[:], s12[:], 1e-8)
            rcpd = rtmp.tile([128, 1], f32, tag="rcpd")
            nc.vector.reciprocal(rcpd[:], s12[:])
            cw = rtmp.tile([128, E], f32, tag="cw")
            nc.vector.tensor_scalar_mul(cw[:], psel[:], rcpd[:])

            # positions within expert buckets: inclusive cumsum + carry
            cum_ps = rpsum.tile([128, E], f32, tag="cum")
            nc.tensor.matmul(cum_ps[:], lhsT=lt_sb[:], rhs=mask2[:], start=True, stop=False)
            nc.tensor.matmul(cum_ps[:], lhsT=ones_sb[0:1, :], rhs=carry[:], start=False, stop=True)
            tot_ps = rpsum.tile([1, E], f32, tag="tot")
            nc.tensor.matmul(tot_ps[:], lhsT=ones_sb[:, 0:1], rhs=mask2[:], start=True, stop=True)
            nc.vector.tensor_add(carry[:], carry[:], tot_ps[:])

            pos = rtmp.tile([128, E], f32, tag="pos")
            nc.vector.tensor_sub(pos[:], cum_ps[:], mask2[:])
            if DEBUG:
                nc.sync.dma_start(out=nc._dbg[0][128 * c : 128 * (c + 1), 0:E], in_=mask2[:])
                nc.sync.dma_start(out=nc._dbg[0][128 * c : 128 * (c + 1), E : 2 * E], in_=pos[:])
                nc.sync.dma_start(out=nc._dbg[0][128 * c : 128 * (c + 1), 2 * E : 3 * E], in_=is1[:])
                nc.sync.dma_start(out=nc._dbg[0][128 * c : 128 * (c + 1), 3 * E : 4 * E], in_=l[:])
            s_all = rtmp.tile([128, E], f32, tag="s_all")
            nc.vector.tensor_add(s_all[:], pos[:], ecap_sb[:])
            if DEBUG:
                nc.sync.dma_start(out=nc._dbg[0][128 * c : 128 * (c + 1), 4 * E : 5 * E], in_=s_all[:])
            ovf = rtmp.tile([128, E], f32, tag="ovf")
            nc.vector.tensor_scalar(ovf[:], pos[:], float(C), None, mybir.AluOpType.is_ge)
            # valid = mask2 * (1 - ovf)
            valid = rtmp.tile([128, E], f32, tag="valid")
            nc.vector.scalar_tensor_tensor(
                out=valid[:], in0=ovf[:], scalar=-1.0, in1=mask2[:],
                op0=mybir.AluOpType.add, op1=mybir.AluOpType.mult,
            )
            nc.vector.tensor_scalar_mul(valid[:], valid[:], -1.0)
            cweff = rtmp.tile([128, E], f32, tag="cweff")
            nc.vector.tensor_mul(cweff[:], cw[:], valid[:])

            is2 = rtmp.tile([128, E], f32, tag="is2")
            nc.vector.tensor_sub(is2[:], mask2[:], is1[:])

            # per-k extraction (k=0 -> top-1 expert, k=1 -> top-2 expert)
            for ki, sel in ((0, is1), (1, is2)):
                col = slice(NT * ki + c, NT * ki + c + 1)
                sk = rtmp.tile([128, 1], f32, tag="sk")
                tmp_a = rtmp.tile([128, E], f32, tag="tmp_a")
                nc.vector.tensor_mul(tmp_a[:], s_all[:], sel[:])
                nc.vector.reduce_sum(sk[:], tmp_a[:], axis=mybir.AxisListType.X)
                if DEBUG:
                    nc.sync.dma_start(out=nc._dbg[0][128 * c : 128 * (c + 1), 5 * E + ki : 5 * E + ki + 1], in_=sk[:])
                vk = rtmp.tile([128, 1], f32, tag="vk")
                tmp_b = rtmp.tile([128, E], f32, tag="tmp_b")
                nc.vector.tensor_mul(tmp_b[:], valid[:], sel[:])
                nc.vector.reduce_sum(vk[:], tmp_b[:], axis=mybir.AxisListType.X)
                tmp_c = rtmp.tile([128, E], f32, tag="tmp_c")
                nc.vector.tensor_mul(tmp_c[:], cweff[:], sel[:])
                nc.vector.reduce_sum(vals_cw[:, col], tmp_c[:], axis=mybir.AxisListType.X)

                # wrapped-layout offset: o = ncol*s - (NSLOT-1)*(s >> shift);
                # invalid slots get s += NSLOT*wrap which maps o -> o + NSLOT
                # (> bounds_check, skipped by the scatter). All values stay in
                # the f32-exact integer range.
                for offs_dst, shift, ncol in (
                    (offs_tok_f, 4, float(NSLOT // 16)),
                    (offs_cw_f, 7, float(NSLOT // 128)),
                ):
                    q_skip = float(NSLOT * (1 << shift))
                    t1 = rtmp.tile([128, 1], f32, tag=f"t1_{shift}")
                    nc.vector.tensor_scalar(
                        t1[:], vk[:], -q_skip, q_skip,
                        mybir.AluOpType.mult, mybir.AluOpType.add,
                    )
                    sm = rtmp.tile([128, 1], f32, tag=f"sm_{shift}")
                    nc.vector.tensor_add(sm[:], sk[:], t1[:])
                    sm_i = rtmp.tile([128, 1], i32, tag=f"smi_{shift}")
                    nc.vector.tensor_copy(sm_i[:], sm[:])
                    q_i = rtmp.tile([128, 1], i32, tag=f"qi_{shift}")
                    nc.vector.tensor_scalar(
                        q_i[:], sm_i[:], shift, None, mybir.AluOpType.arith_shift_right
                    )
                    q_f = rtmp.tile([128, 1], f32, tag=f"qf_{shift}")
                    nc.vector.tensor_copy(q_f[:], q_i[:])
                    a_f = rtmp.tile([128, 1], f32, tag=f"af_{shift}")
                    nc.vector.tensor_scalar_mul(a_f[:], sm[:], ncol)
                    nc.vector.scalar_tensor_tensor(
                        out=offs_dst[:, col], in0=q_f[:], scalar=-float(NSLOT - 1),
                        in1=a_f[:],
                        op0=mybir.AluOpType.mult, op1=mybir.AluOpType.add,
                    )

        # scatter dispatch tables to DRAM
        offs_tok_i = rbig.tile([128, 2 * NT], i32)
        nc.vector.tensor_copy(offs_tok_i[:], offs_tok_f[:])
        if DEBUG:
            nc.sync.dma_start(out=nc._dbg[1][:, 0 : 2 * NT], in_=offs_tok_f[:])
            nc.sync.dma_start(out=nc._dbg[1][:, 2 * NT : 4 * NT], in_=vals_cw[:])
            nc.sync.dma_start(out=nc._dbg[1][:, 4 * NT : 6 * NT], in_=offs_cw_f[:])
        offs_cw_i = rbig.tile([128, 2 * NT], i32)
        nc.vector.tensor_copy(offs_cw_i[:], offs_cw_f[:])
        # column-wise scatters: [128,1] per-partition offsets pair unambiguously
        for j in range(2 * NT):
            nc.gpsimd.indirect_dma_start(
                out=tokdram[:, :].rearrange("a b -> (a b)")[:, None],
                out_offset=bass.IndirectOffsetOnAxis(ap=offs_tok_i[:, j : j + 1], axis=0),
                in_=tokid_sb[:, j : j + 1],
                in_offset=None,
                bounds_check=NSLOT - 1,
                oob_is_err=False,
            )
            nc.gpsimd.indirect_dma_start(
                out=cwdram[:, :].rearrange("a b -> (a b)")[:, None],
                out_offset=bass.IndirectOffsetOnAxis(ap=offs_cw_i[:, j : j + 1], axis=0),
                in_=vals_cw[:, j : j + 1],
                in_offset=None,
                bounds_check=NSLOT - 1,
                oob_is_err=False,
            )

        # aux partial sums -> [1,16]
        aux_sb = rbig.tile([1, 16], f32)
        nc.vector.memset(aux_sb[:], 0.0)
        us_ps = rpsum.tile([1, E], f32, tag="tot")
        nc.tensor.matmul(us_ps[:], lhsT=ones_sb[:, 0:1], rhs=usage_acc[:], start=True, stop=True)
        nc.vector.tensor_copy(aux_sb[0:1, 0:E], us_ps[:])
        zs_ps = rpsum.tile([1, 1], f32, tag="tot")
        nc.tensor.matmul(zs_ps[:], lhsT=ones_sb[:, 0:1], rhs=zsq_acc[:], start=True, stop=True)
        nc.vector.tensor_copy(aux_sb[0:1, E : E + 1], zs_ps[:])
        nc.sync.dma_start(out=aux[:], in_=aux_sb[:])

        # load dispatch tables back (wrapped idx layout, replicated x8)
        for g in range(8):
            nc.sync.dma_start(out=idxs_sb[16 * g : 16 * (g + 1), :], in_=tokdram[:])
        nc.sync.dma_start(out=cw_sb[:], in_=cwdram[:])
        if DEBUG:
            idxf = rbig.tile([128, NSLOT // 16], f32)
            nc.vector.tensor_copy(idxf[:], idxs_sb[:])
            nc.sync.dma_start(out=nc._dbg[4][:], in_=idxf[:])
            nc.sync.dma_start(out=nc._dbg[5][:], in_=cw_sb[:])


def ffn_phase(nc, tc, xbf, w1, w2, out, b1_sb, b2_sb, onesbf_sb, idxs_sb, cw_sb):
    with (
        tc.tile_pool(name="w1p", bufs=2) as w1p,
        tc.tile_pool(name="w2p", bufs=3) as w2p,
        tc.tile_pool(name="xtp", bufs=2) as xtp,
        tc.tile_pool(name="hp", bufs=2) as hp,
        tc.tile_pool(name="yp", bufs=3) as yp,
        tc.tile_pool(name="ps1", bufs=2, space="PSUM") as ps1p,
        tc.tile_pool(name="ps2", bufs=1, space="PSUM") as ps2p,
    ):
        for e in range(E):
            xt_e = xtp.tile([128, KD, C], bf16, tag="xt")
            nc.gpsimd.dma_gather(
                out_ap=xt_e[:],
                in_ap=xbf[:, :],
                idxs_ap=idxs_sb[:, (C // 16) * e : (C // 16) * (e + 1)],
                num_idxs=C,
                num_idxs_reg=C,
                elem_size=D,
                transpose=True,
            )
            if DEBUG:
                nc.sync.dma_start(out=nc._dbg[2][:, e, :, :], in_=xt_e[:])
            w1_e = w1p.tile([128, KD, F], bf16, tag="w1")
            nc.sync.dma_start(
                out=w1_e[:], in_=w1[e, :, :].rearrange("(k p) f -> p k f", p=128)
            )
            h_e = hp.tile([128, MF, C], bf16, tag="h")
            for m in range(MF):
                ps = ps1p.tile([128, C], f32, tag="ps1")
                for k in range(KD):
                    nc.tensor.matmul(
                        ps[:],
                        lhsT=w1_e[:, k, 128 * m : 128 * (m + 1)],
                        rhs=xt_e[:, k, :],
                        start=(k == 0),
                        stop=(k == KD - 1),
                    )
                nc.scalar.activation(
                    h_e[:, m, :], ps[:], mybir.ActivationFunctionType.Gelu,
                    bias=b1_sb[:, e, m : m + 1], scale=1.0,
                )
            if DEBUG:
                nc.sync.dma_start(out=nc._dbg[3][:, e, :, :], in_=h_e[:, 0:4, :])

            # mm2: k-outer over F, 6 live PSUM banks (3 slot-tiles x 2 halves)
            ps2 = [
                [
                    ps2p.tile(
                        [128, D // 2], f32,
                        tag=f"ps2_{mt}_{nh}", name=f"ps2_{mt}_{nh}",
                    )
                    for nh in range(2)
                ]
                for mt in range(CT)
            ]
            for k in range(MF):
                w2_k = w2p.tile([128, D], bf16, tag="w2")
                nc.sync.dma_start(out=w2_k[:], in_=w2[e, 128 * k : 128 * (k + 1), :])
                for mt in range(CT):
                    for nh in range(2):
                        nc.tensor.matmul(
                            ps2[mt][nh][:],
                            lhsT=h_e[:, k, 128 * mt : 128 * (mt + 1)],
                            rhs=w2_k[:, (D // 2) * nh : (D // 2) * (nh + 1)],
                            start=(k == 0),
                            stop=False,
                        )
            for mt in range(CT):
                for nh in range(2):
                    nc.tensor.matmul(
                        ps2[mt][nh][:],
                        lhsT=onesbf_sb[:],
                        rhs=b2_sb[0:1, e, (D // 2) * nh : (D // 2) * (nh + 1)],
                        start=False,
                        stop=True,
                    )
            for mt in range(CT):
                y_sb = yp.tile([128, 1, D], f32, tag="y")
                slot_col = CT * e + mt
                for nh in range(2):
                    nc.scalar.activation(
                        y_sb[:, 0, (D // 2) * nh : (D // 2) * (nh + 1)],
                        ps2[mt][nh][:],
                        mybir.ActivationFunctionType.Copy,
                        bias=0.0,
                        scale=cw_sb[:, slot_col : slot_col + 1],
                    )
                nc.gpsimd.dma_scatter_add(
                    out_ap=out[:, :],
                    in_ap=y_sb[:],
                    idxs_ap=idxs_sb[:, (C // 16) * e + 8 * mt : (C // 16) * e + 8 * (mt + 1)],
                    num_idxs=128,
                    num_idxs_reg=128,
                    elem_size=D,
                )


def build_program():
    nc = bacc.Bacc(None, target_bir_lowering=False)

    # ---- DRAM I/O ----
    xT = nc.declare_dram_parameter("xT", [D, T], f32, isOutput=False)
    xbf = nc.declare_dram_parameter("xbf", [T + 1, D], bf16, isOutput=False)
    wr = nc.declare_dram_parameter("wr", [128, KD, E], f32, isOutput=False)
    w1 = nc.declare_dram_parameter("w1", [E, D, F], bf16, isOutput=False)
    w2 = nc.declare_dram_parameter("w2", [E, F, D], bf16, isOutput=False)
    b1 = nc.declare_dram_parameter("b1", [128, E, MF], f32, isOutput=False)
    b2 = nc.declare_dram_parameter("b2", [E, D], bf16, isOutput=False)
    lt = nc.declare_dram_parameter("lt", [128, 128], f32, isOutput=False)
    ecap = nc.declare_dram_parameter("ecap", [128, E], f32, isOutput=False)
    tokid = nc.declare_dram_parameter("tokid", [128, 2 * NT], i16, isOutput=False)
    tokfill = nc.declare_dram_parameter("tokfill", [16, NSLOT // 16], i16, isOutput=False)
    ones = nc.declare_dram_parameter("ones", [128, 128], f32, isOutput=False)
    onesbf = nc.declare_dram_parameter("onesbf", [1, 128], bf16, isOutput=False)

    out = nc.declare_dram_parameter("out", [T + 1, D], f32, isOutput=True)
    aux = nc.declare_dram_parameter("aux", [1, 16], f32, isOutput=True)

    if DEBUG:
        nc._dbg = [
            nc.declare_dram_parameter("dbg", [T, 6 * E], f32, isOutput=True),
            nc.declare_dram_parameter("dbg2", [128, 6 * NT], f32, isOutput=True),
            nc.declare_dram_parameter("dbgxt", [128, E, KD, C], bf16, isOutput=True),
            nc.declare_dram_parameter("dbgh", [128, E, 4, C], bf16, isOutput=True),
            nc.declare_dram_parameter("dbgidx", [128, NSLOT // 16], f32, isOutput=True),
            nc.declare_dram_parameter("dbgcw", [128, NSLOT // 128], f32, isOutput=True),
        ]

    # ---- DRAM internal ----
    tokdram = nc.dram_tensor("tokdram", [16, NSLOT // 16], i16)
    cwdram = nc.dram_tensor("cwdram", [128, NSLOT // 128], f32)

    with tile.TileContext(nc) as tc:
        with (
            tc.tile_pool(name="const", bufs=1) as constp,
            tc.tile_pool(name="routr", bufs=1) as routr,
        ):
            # ---------- phase 0: constants + init ----------
            lt_sb = constp.tile([128, 128], f32)
            nc.sync.dma_start(out=lt_sb[:], in_=lt[:])
            ones_sb = constp.tile([128, 128], f32)
            nc.sync.dma_start(out=ones_sb[:], in_=ones[:])
            onesbf_sb = constp.tile([1, 128], bf16)
            nc.sync.dma_start(out=onesbf_sb[:], in_=onesbf[:])
            ecap_sb = constp.tile([128, E], f32)
            nc.sync.dma_start(out=ecap_sb[:], in_=ecap[:])
            tokid_sb = constp.tile([128, 2 * NT], i16)
            nc.sync.dma_start(out=tokid_sb[:], in_=tokid[:])
            wr_sb = constp.tile([128, KD, E], f32)
            nc.sync.dma_start(out=wr_sb[:], in_=wr[:])
            b1_sb = constp.tile([128, E, MF], f32)
            nc.sync.dma_start(out=b1_sb[:], in_=b1[:])
            b2_sb = constp.tile([1, E, D], bf16)
            nc.sync.dma_start(out=b2_sb[:], in_=b2[None, :, :])

            # default-fill dispatch tables
            nc.sync.dma_start(out=tokdram[:], in_=tokfill[:])
            zero_sb = constp.tile([128, D], f32)
            nc.vector.memset(zero_sb[:], 0.0)
            # zero the output accumulator and the cw table
            for c in range(NT):
                nc.sync.dma_start(out=out[128 * c : 128 * (c + 1), :], in_=zero_sb[:])
            nc.sync.dma_start(out=out[T : T + 1, :], in_=zero_sb[0:1, :])
            nc.sync.dma_start(out=cwdram[:], in_=zero_sb[:, : NSLOT // 128])

            # dispatch tables that outlive the router scope
            idxs_sb = routr.tile([128, NSLOT // 16], i16)
            cw_sb = routr.tile([128, NSLOT // 128], f32)

            router_phase(
                nc, tc, xT, tokdram, cwdram, aux,
                lt_sb, ones_sb, ecap_sb, tokid_sb, wr_sb, idxs_sb, cw_sb,
            )
            ffn_phase(
                nc, tc, xbf, w1, w2, out,
                b1_sb, b2_sb, onesbf_sb, idxs_sb, cw_sb,
            )

    nc.compile()
    return nc


_NC_CACHE = None


def _get_nc():
    global _NC_CACHE
    if _NC_CACHE is None:
        _NC_CACHE = build_program()
    return _NC_CACHE


def make_in_maps(x, Wr, W1, b1, W2, b2):
    xf = np.ascontiguousarray(np.asarray(x, np.float32).reshape(B * S, D))
    W1bf = np.asarray(W1, np.float32).astype(BF16)
    W2bf = np.asarray(W2, np.float32).astype(BF16)
    b2bf = np.asarray(b2, np.float32).astype(BF16)
    wr_p = np.ascontiguousarray(
        np.asarray(Wr, np.float32).reshape(KD, 128, E).transpose(1, 0, 2)
    )
    b1_p = np.ascontiguousarray(
        np.asarray(b1, np.float32).reshape(E, MF, 128).transpose(2, 0, 1)
    )
    lt_c = np.triu(np.ones((128, 128), np.float32))
    ecap_c = np.broadcast_to((np.arange(E) * C).astype(np.float32), (128, E)).copy()
    tokid_c = (
        (np.arange(2 * NT)[None, :] % NT) * 128 + np.arange(128)[:, None]
    ).astype(np.int16)
    tokfill_c = np.full((16, NSLOT // 16), DUMP, np.int16)
    ones_c = np.ones((128, 128), np.float32)
    onesbf_c = np.ones((1, 128), BF16)

    shared = dict(
        wr=wr_p, w1=W1bf, w2=W2bf, b1=b1_p, b2=b2bf, lt=lt_c, ecap=ecap_c,
        tokid=tokid_c, tokfill=tokfill_c, ones=ones_c, onesbf=onesbf_c,
    )
    in_maps = []
    for c in range(NCORES):
        xs = xf[T * c : T * (c + 1)]
        xbf_c = np.zeros((T + 1, D), BF16)
        xbf_c[:T] = xs.astype(BF16)
        m = dict(shared)
        m["xT"] = np.ascontiguousarray(xs.T)
        m["xbf"] = xbf_c
        in_maps.append(m)
    return in_maps


def finish(results):
    outp = np.empty((B * S, D), np.float32)
    usage_sum = np.zeros(E, np.float64)
    zsq_sum = 0.0
    for c in range(NCORES):
        outp[T * c : T * (c + 1)] = results[c]["out"][:T]
        a = results[c]["aux"][0]
        usage_sum += a[:E].astype(np.float64)
        zsq_sum += float(a[E])
    n = float(B * S)
    usage = (usage_sum / n).astype(np.float32)
    mean_u = np.float32(usage.mean())
    var_u = np.float32(np.mean((usage - mean_u) ** 2))
    lb = var_u / (mean_u * mean_u + np.float32(1e-8)) * np.float32(E) * np.float32(0.01)
    z = np.float32(zsq_sum / n) * np.float32(0.001)
    return outp.reshape(B, S, D), np.float32(lb), np.float32(z)


def kernel(x, Wr, W1, b1, W2, b2):
    nc = _get_nc()
    in_maps = make_in_maps(x, Wr, W1, b1, W2, b2)
    res = run_bass_kernel_spmd(nc, in_maps, list(range(NCORES)))
    return finish(res.results)


# revision 19
# speedup vs baseline: 1.0598x; 1.0598x over previous
"""MoE (B=4,S=2048,D=768,F=3072,E=8,K=2) on 8 TRN2 NeuronCores.

Strategy: data-parallel over tokens (1024 tokens/core), sparse top-2 expert
compute with capacity-padded per-expert buckets (C=384 slots/expert/core).
Router, top-2 selection, dispatch-index build, FFN, and weighted combine all
run on-device; the host only shards inputs, sums 8 tiny per-core partial
vectors for the aux losses, and reassembles the output.
"""

import numpy as np
import ml_dtypes

import concourse.bass as bass
import concourse.mybir as mybir
import concourse.tile as tile
from concourse import bacc
from concourse.bass_utils import run_bass_kernel_spmd

BF16 = ml_dtypes.bfloat16

B, S, D, F, E, K = 4, 2048, 768, 3072, 8, 2
NCORES = 8
T = (B * S) // NCORES          # 1024 tokens per core
NT = T // 128                  # 8 token tiles
C = 384                        # per-expert capacity (multiple of 128)
NSLOT = E * C                  # 3072 dispatch slots
DUMP = T                       # dump row index (row T of xbf / out)
KD = D // 128                  # 6 k-tiles over D
MF = F // 128                  # 24 m-tiles over F
CT = C // 128                  # 3 slot tiles per expert
BIGOFF = 16384.0               # scatter-skip offset (> bounds_check, f32-exact range)

DEBUG = False

f32 = mybir.dt.float32
bf16 = mybir.dt.bfloat16
i16 = mybir.dt.int16
i32 = mybir.dt.int32


def router_phase(nc, tc, xT, tokdram, cwdram, aux,
                 lt_sb, ones_sb, ecap_sb, tokid_sb, wr_sb, idxs_sb, cw_sb):
    with (
        tc.tile_pool(name="rbig", bufs=1) as rbig,
        tc.tile_pool(name="rpsum", bufs=2, space="PSUM") as rpsum,
        tc.tile_pool(name="rtmp", bufs=2) as rtmp,
    ):
        xT_sb = rbig.tile([128, KD, T], f32)
        nc.sync.dma_start(out=xT_sb[:], in_=xT[:, :].rearrange("(k p) t -> p k t", p=128))

        carry = rbig.tile([1, E], f32)
        nc.vector.memset(carry[:], 0.0)
        usage_acc = rbig.tile([128, E], f32)
        nc.vector.memset(usage_acc[:], 0.0)
        zsq_acc = rbig.tile([128, 1], f32)
        nc.vector.memset(zsq_acc[:], 0.0)
        offs_tok_f = rbig.tile([128, 2 * NT], f32)
        offs_cw_f = rbig.tile([128, 2 * NT], f32)
        vals_cw = rbig.tile([128, 2 * NT], f32)

        for c in range(NT):
            tok_sl = slice(128 * c, 128 * (c + 1))
            lg_ps = rpsum.tile([128, E], f32, tag="lg")
            for k in range(KD):
                nc.tensor.matmul(
                    lg_ps[:], lhsT=xT_sb[:, k, tok_sl], rhs=wr_sb[:, k, :],
                    start=(k == 0), stop=(k == KD - 1),
                )
            l = rtmp.tile([128, E], f32, tag="l")
            nc.scalar.copy(out=l[:], in_=lg_ps[:])

            m1 = rtmp.tile([128, 1], f32, tag="m1")
            nc.vector.reduce_max(m1[:], l[:], axis=mybir.AxisListType.X)
            is1 = rtmp.tile([128, E], f32, tag="is1")
            nc.vector.tensor_scalar(is1[:], l[:], m1[:], None, mybir.AluOpType.is_ge)
            # mask out the top-1 and find the second max
            lmask = rtmp.tile([128, E], f32, tag="lmask")
            nc.vector.scalar_tensor_tensor(
                out=lmask[:], in0=is1[:], scalar=-1e9, in1=l[:],
                op0=mybir.AluOpType.mult, op1=mybir.AluOpType.add,
            )
            m2 = rtmp.tile([128, 1], f32, tag="m2")
            nc.vector.reduce_max(m2[:], lmask[:], axis=mybir.AxisListType.X)

            # softmax (full 8) + logsumexp
            negm1 = rtmp.tile([128, 1], f32, tag="negm1")
            nc.vector.tensor_scalar_mul(negm1[:], m1[:], -1.0)
            ex = rtmp.tile([128, E], f32, tag="ex")
            sume = rtmp.tile([128, 1], f32, tag="sume")
            nc.scalar.activation(
                ex[:], l[:], mybir.ActivationFunctionType.Exp,
                bias=negm1[:], scale=1.0, accum_out=sume[:],
            )
            rcps = rtmp.tile([128, 1], f32, tag="rcps")
            nc.vector.reciprocal(rcps[:], sume[:])
            p = rtmp.tile([128, E], f32, tag="p")
            nc.vector.tensor_scalar_mul(p[:], ex[:], rcps[:])
            nc.vector.tensor_add(usage_acc[:], usage_acc[:], p[:])
            lns = rtmp.tile([128, 1], f32, tag="lns")
            nc.scalar.activation(lns[:], sume[:], mybir.ActivationFunctionType.Ln)
            z = rtmp.tile([128, 1], f32, tag="z")
            nc.vector.tensor_add(z[:], lns[:], m1[:])
            z2 = rtmp.tile([128, 1], f32, tag="z2")
            nc.vector.tensor_mul(z2[:], z[:], z[:])
            nc.vector.tensor_add(zsq_acc[:], zsq_acc[:], z2[:])

            # top-2 combine weights
            mask2 = rtmp.tile([128, E], f32, tag="mask2")
            nc.vector.tensor_scalar(mask2[:], l[:], m2[:], None, mybir.AluOpType.is_ge)
            psel = rtmp.tile([128, E], f32, tag="psel")
            nc.vector.tensor_mul(psel[:], p[:], mask2[:])
            s12 = rtmp.tile([128, 1], f32, tag="s12")
            nc.vector.reduce_sum(s12[:], psel[:], axis=mybir.AxisListType.X)
            nc.vector.tensor_scalar_add(s12[:], s12[:], 1e-8)
            rcpd = rtmp.tile([128, 1], f32, tag="rcpd")
            nc.vector.reciprocal(rcpd[:], s12[:])
            cw = rtmp.tile([128, E], f32, tag="cw")
            nc.vector.tensor_scalar_mul(cw[:], psel[:], rcpd[:])

            # positions within expert buckets: inclusive cumsum + carry
            cum_ps = rpsum.tile([128, E], f32, tag="cum")
            nc.tensor.matmul(cum_ps[:], lhsT=lt_sb[:], rhs=mask2[:], start=True, stop=False)
            nc.tensor.matmul(cum_ps[:], lhsT=ones_sb[0:1, :], rhs=carry[:], start=False, stop=True)
            tot_ps = rpsum.tile([1, E], f32, tag="tot")
            nc.tensor.matmul(tot_ps[:], lhsT=ones_sb[:, 0:1], rhs=mask2[:], start=True, stop=True)
            nc.vector.tensor_add(carry[:], carry[:], tot_ps[:])

            pos = rtmp.tile([128, E], f32, tag="pos")
            nc.vector.tensor_sub(pos[:], cum_ps[:], mask2[:])
            if DEBUG:
                nc.sync.dma_start(out=nc._dbg[0][128 * c : 128 * (c + 1), 0:E], in_=mask2[:])
                nc.sync.dma_start(out=nc._dbg[0][128 * c : 128 * (c + 1), E : 2 * E], in_=pos[:])
                nc.sync.dma_start(out=nc._dbg[0][128 * c : 128 * (c + 1), 2 * E : 3 * E], in_=is1[:])
                nc.sync.dma_start(out=nc._dbg[0][128 * c : 128 * (c + 1), 3 * E : 4 * E], in_=l[:])
            s_all = rtmp.tile([128, E], f32, tag="s_all")
            nc.vector.tensor_add(s_all[:], pos[:], ecap_sb[:])
            if DEBUG:
                nc.sync.dma_start(out=nc._dbg[0][128 * c : 128 * (c + 1), 4 * E : 5 * E], in_=s_all[:])
            ovf = rtmp.tile([128, E], f32, tag="ovf")
            nc.vector.tensor_scalar(ovf[:], pos[:], float(C), None, mybir.AluOpType.is_ge)
            # valid = mask2 * (1 - ovf)
            valid = rtmp.tile([128, E], f32, tag="valid")
            nc.vector.scalar_tensor_tensor(
                out=valid[:], in0=ovf[:], scalar=-1.0, in1=mask2[:],
                op0=mybir.AluOpType.add, op1=mybir.AluOpType.mult,
            )
            nc.vector.tensor_scalar_mul(valid[:], valid[:], -1.0)
            cweff = rtmp.tile([128, E], f32, tag="cweff")
            nc.vector.tensor_mul(cweff[:], cw[:], valid[:])

            is2 = rtmp.tile([128, E], f32, tag="is2")
            nc.vector.tensor_sub(is2[:], mask2[:], is1[:])

            # per-k extraction (k=0 -> top-1 expert, k=1 -> top-2 expert)
            for ki, sel in ((0, is1), (1, is2)):
                col = slice(NT * ki + c, NT * ki + c + 1)
                sk = rtmp.tile([128, 1], f32, tag="sk")
                tmp_a = rtmp.tile([128, E], f32, tag="tmp_a")
                nc.vector.tensor_mul(tmp_a[:], s_all[:], sel[:])
                nc.vector.reduce_sum(sk[:], tmp_a[:], axis=mybir.AxisListType.X)
                if DEBUG:
                    nc.sync.dma_start(out=nc._dbg[0][128 * c : 128 * (c + 1), 5 * E + ki : 5 * E + ki + 1], in_=sk[:])
                vk = rtmp.tile([128, 1], f32, tag="vk")
                tmp_b = rtmp.tile([128, E], f32, tag="tmp_b")
                nc.vector.tensor_mul(tmp_b[:], valid[:], sel[:])
                nc.vector.reduce_sum(vk[:], tmp_b[:], axis=mybir.AxisListType.X)
                tmp_c = rtmp.tile([128, E], f32, tag="tmp_c")
                nc.vector.tensor_mul(tmp_c[:], cweff[:], sel[:])
                nc.vector.reduce_sum(vals_cw[:, col], tmp_c[:], axis=mybir.AxisListType.X)

                # wrapped-layout offset: o = ncol*s - (NSLOT-1)*(s >> shift);
                # invalid slots get s += NSLOT*wrap which maps o -> o + NSLOT
                # (> bounds_check, skipped by the scatter). All values stay in
                # the f32-exact integer range.
                for offs_dst, shift, ncol in (
                    (offs_tok_f, 4, float(NSLOT // 16)),
                    (offs_cw_f, 7, float(NSLOT // 128)),
                ):
                    q_skip = float(NSLOT * (1 << shift))
                    t1 = rtmp.tile([128, 1], f32, tag=f"t1_{shift}")
                    nc.vector.tensor_scalar(
                        t1[:], vk[:], -q_skip, q_skip,
                        mybir.AluOpType.mult, mybir.AluOpType.add,
                    )
                    sm = rtmp.tile([128, 1], f32, tag=f"sm_{shift}")
                    nc.vector.tensor_add(sm[:], sk[:], t1[:])
                    sm_i = rtmp.tile([128, 1], i32, tag=f"smi_{shift}")
                    nc.vector.tensor_copy(sm_i[:], sm[:])
                    q_i = rtmp.tile([128, 1], i32, tag=f"qi_{shift}")
                    nc.vector.tensor_scalar(
                        q_i[:], sm_i[:], shift, None, mybir.AluOpType.arith_shift_right
                    )
                    q_f = rtmp.tile([128, 1], f32, tag=f"qf_{shift}")
                    nc.vector.tensor_copy(q_f[:], q_i[:])
                    a_f = rtmp.tile([128, 1], f32, tag=f"af_{shift}")
                    nc.vector.tensor_scalar_mul(a_f[:], sm[:], ncol)
                    of = rtmp.tile([128, 1], f32, tag=f"of_{shift}")
                    nc.vector.scalar_tensor_tensor(
                        out=of[:], in0=q_f[:], scalar=-float(NSLOT - 1),
                        in1=a_f[:],
                        op0=mybir.AluOpType.mult, op1=mybir.AluOpType.add,
                    )
                    oi = rtmp.tile([128, 1], i32, tag=f"oi_{shift}")
                    nc.vector.tensor_copy(oi[:], of[:])
                    if DEBUG:
                        nc.vector.tensor_copy(offs_dst[:, col], of[:])
                    if shift == 4:
                        nc.gpsimd.indirect_dma_start(
                            out=tokdram[:, :].rearrange("a b -> (a b)")[:, None],
                            out_offset=bass.IndirectOffsetOnAxis(ap=oi[:], axis=0),
                            in_=tokid_sb[:, NT * ki + c : NT * ki + c + 1],
                            in_offset=None,
                            bounds_check=NSLOT - 1,
                            oob_is_err=False,
                        )
                    else:
                        nc.gpsimd.indirect_dma_start(
                            out=cwdram[:, :].rearrange("a b -> (a b)")[:, None],
                            out_offset=bass.IndirectOffsetOnAxis(ap=oi[:], axis=0),
                            in_=vals_cw[:, col],
                            in_offset=None,
                            bounds_check=NSLOT - 1,
                            oob_is_err=False,
                        )

        if DEBUG:
            nc.sync.dma_start(out=nc._dbg[1][:, 0 : 2 * NT], in_=offs_tok_f[:])
            nc.sync.dma_start(out=nc._dbg[1][:, 2 * NT : 4 * NT], in_=vals_cw[:])
            nc.sync.dma_start(out=nc._dbg[1][:, 4 * NT : 6 * NT], in_=offs_cw_f[:])
        # aux partial sums -> [1,16]
        aux_sb = rbig.tile([1, 16], f32)
        nc.vector.memset(aux_sb[:], 0.0)
        us_ps = rpsum.tile([1, E], f32, tag="tot")
        nc.tensor.matmul(us_ps[:], lhsT=ones_sb[:, 0:1], rhs=usage_acc[:], start=True, stop=True)
        nc.vector.tensor_copy(aux_sb[0:1, 0:E], us_ps[:])
        zs_ps = rpsum.tile([1, 1], f32, tag="tot")
        nc.tensor.matmul(zs_ps[:], lhsT=ones_sb[:, 0:1], rhs=zsq_acc[:], start=True, stop=True)
        nc.vector.tensor_copy(aux_sb[0:1, E : E + 1], zs_ps[:])
        nc.sync.dma_start(out=aux[:], in_=aux_sb[:])

        # load dispatch tables back (wrapped idx layout, replicated x8)
        for g in range(8):
            nc.sync.dma_start(out=idxs_sb[16 * g : 16 * (g + 1), :], in_=tokdram[:])
        nc.sync.dma_start(out=cw_sb[:], in_=cwdram[:])
        if DEBUG:
            idxf = rbig.tile([128, NSLOT // 16], f32)
            nc.vector.tensor_copy(idxf[:], idxs_sb[:])
            nc.sync.dma_start(out=nc._dbg[4][:], in_=idxf[:])
            nc.sync.dma_start(out=nc._dbg[5][:], in_=cw_sb[:])


def ffn_phase(nc, tc, xbf, w1, w2, out, b1_sb, b2_sb, onesbf_sb, idxs_sb, cw_sb):
    with (
        tc.tile_pool(name="w1p", bufs=2) as w1p,
        tc.tile_pool(name="w2p", bufs=3) as w2p,
        tc.tile_pool(name="xtp", bufs=1) as xtp,
        tc.tile_pool(name="hp", bufs=2) as hp,
        tc.tile_pool(name="yp", bufs=3) as yp,
        tc.tile_pool(name="ps1", bufs=2, space="PSUM") as ps1p,
        tc.tile_pool(name="ps2", bufs=1, space="PSUM") as ps2p,
    ):
        xt_all = []
        for e in range(E):
            xt_e = xtp.tile([128, KD, C], bf16, tag=f"xt{e}", name=f"xt{e}")
            nc.gpsimd.dma_gather(
                out_ap=xt_e[:],
                in_ap=xbf[:, :],
                idxs_ap=idxs_sb[:, (C // 16) * e : (C // 16) * (e + 1)],
                num_idxs=C,
                num_idxs_reg=C,
                elem_size=D,
                transpose=True,
            )
            xt_all.append(xt_e)
        for e in range(E):
            xt_e = xt_all[e]
            if DEBUG:
                nc.sync.dma_start(out=nc._dbg[2][:, e, :, :], in_=xt_e[:])
            w1_e = w1p.tile([128, KD, F], bf16, tag="w1")
            nc.sync.dma_start(
                out=w1_e[:], in_=w1[e, :, :].rearrange("(k p) f -> p k f", p=128)
            )
            h_e = hp.tile([128, MF, C], bf16, tag="h")
            for m in range(MF):
                ps = ps1p.tile([128, C], f32, tag="ps1")
                for k in range(KD):
                    nc.tensor.matmul(
                        ps[:],
                        lhsT=w1_e[:, k, 128 * m : 128 * (m + 1)],
                        rhs=xt_e[:, k, :],
                        start=(k == 0),
                        stop=(k == KD - 1),
                    )
                nc.scalar.activation(
                    h_e[:, m, :], ps[:], mybir.ActivationFunctionType.Gelu,
                    bias=b1_sb[:, e, m : m + 1], scale=1.0,
                )
            if DEBUG:
                nc.sync.dma_start(out=nc._dbg[3][:, e, :, :], in_=h_e[:, 0:4, :])

            # mm2: k-outer over F, 6 live PSUM banks (3 slot-tiles x 2 halves)
            ps2 = [
                [
                    ps2p.tile(
                        [128, D // 2], f32,
                        tag=f"ps2_{mt}_{nh}", name=f"ps2_{mt}_{nh}",
                    )
                    for nh in range(2)
                ]
                for mt in range(CT)
            ]
            for k in range(MF):
                w2_k = w2p.tile([128, D], bf16, tag="w2")
                nc.sync.dma_start(out=w2_k[:], in_=w2[e, 128 * k : 128 * (k + 1), :])
                for mt in range(CT):
                    for nh in range(2):
                        nc.tensor.matmul(
                            ps2[mt][nh][:],
                            lhsT=h_e[:, k, 128 * mt : 128 * (mt + 1)],
                            rhs=w2_k[:, (D // 2) * nh : (D // 2) * (nh + 1)],
                            start=(k == 0),
                            stop=False,
                        )
            for mt in range(CT):
                for nh in range(2):
                    nc.tensor.matmul(
                        ps2[mt][nh][:],
                        lhsT=onesbf_sb[:],
                        rhs=b2_sb[0:1, e, (D // 2) * nh : (D // 2) * (nh + 1)],
                        start=False,
                        stop=True,
                    )
            for mt in range(CT):
                y_sb = yp.tile([128, 1, D], f32, tag="y")
                slot_col = CT * e + mt
                for nh in range(2):
                    nc.vector.tensor_scalar_mul(
                        y_sb[:, 0, (D // 2) * nh : (D // 2) * (nh + 1)],
                        ps2[mt][nh][:],
                        cw_sb[:, slot_col : slot_col + 1],
                    )
                nc.gpsimd.dma_scatter_add(
                    out_ap=out[:, :],
                    in_ap=y_sb[:],
                    idxs_ap=idxs_sb[:, (C // 16) * e + 8 * mt : (C // 16) * e + 8 * (mt + 1)],
                    num_idxs=128,
                    num_idxs_reg=128,
                    elem_size=D,
                )


def build_program():
    nc = bacc.Bacc(None, target_bir_lowering=False)

    # ---- DRAM I/O ----
    xT = nc.declare_dram_parameter("xT", [D, T], f32, isOutput=False)
    xbf = nc.declare_dram_parameter("xbf", [T + 1, D], bf16, isOutput=False)
    wr = nc.declare_dram_parameter("wr", [128, KD, E], f32, isOutput=False)
    w1 = nc.declare_dram_parameter("w1", [E, D, F], bf16, isOutput=False)
    w2 = nc.declare_dram_parameter("w2", [E, F, D], bf16, isOutput=False)
    b1 = nc.declare_dram_parameter("b1", [128, E, MF], f32, isOutput=False)
    b2 = nc.declare_dram_parameter("b2", [E, D], bf16, isOutput=False)
    lt = nc.declare_dram_parameter("lt", [128, 128], f32, isOutput=False)
    ecap = nc.declare_dram_parameter("ecap", [128, E], f32, isOutput=False)
    tokid = nc.declare_dram_parameter("tokid", [128, 2 * NT], i16, isOutput=False)
    tokfill = nc.declare_dram_parameter("tokfill", [16, NSLOT // 16], i16, isOutput=False)
    ones = nc.declare_dram_parameter("ones", [128, 128], f32, isOutput=False)
    onesbf = nc.declare_dram_parameter("onesbf", [1, 128], bf16, isOutput=False)

    out = nc.declare_dram_parameter("out", [T + 1, D], f32, isOutput=True)
    aux = nc.declare_dram_parameter("aux", [1, 16], f32, isOutput=True)

    if DEBUG:
        nc._dbg = [
            nc.declare_dram_parameter("dbg", [T, 6 * E], f32, isOutput=True),
            nc.declare_dram_parameter("dbg2", [128, 6 * NT], f32, isOutput=True),
            nc.declare_dram_parameter("dbgxt", [128, E, KD, C], bf16, isOutput=True),
            nc.declare_dram_parameter("dbgh", [128, E, 4, C], bf16, isOutput=True),
            nc.declare_dram_parameter("dbgidx", [128, NSLOT // 16], f32, isOutput=True),
            nc.declare_dram_parameter("dbgcw", [128, NSLOT // 128], f32, isOutput=True),
        ]

    # ---- DRAM internal ----
    tokdram = nc.dram_tensor("tokdram", [16, NSLOT // 16], i16)
    cwdram = nc.dram_tensor("cwdram", [128, NSLOT // 128], f32)

    with tile.TileContext(nc) as tc:
        with (
            tc.tile_pool(name="const", bufs=1) as constp,
            tc.tile_pool(name="routr", bufs=1) as routr,
        ):
            # ---------- phase 0: constants + init ----------
            lt_sb = constp.tile([128, 128], f32)
            nc.sync.dma_start(out=lt_sb[:], in_=lt[:])
            ones_sb = constp.tile([128, 128], f32)
            nc.sync.dma_start(out=ones_sb[:], in_=ones[:])
            onesbf_sb = constp.tile([1, 128], bf16)
            nc.sync.dma_start(out=onesbf_sb[:], in_=onesbf[:])
            ecap_sb = constp.tile([128, E], f32)
            nc.sync.dma_start(out=ecap_sb[:], in_=ecap[:])
            tokid_sb = constp.tile([128, 2 * NT], i16)
            nc.sync.dma_start(out=tokid_sb[:], in_=tokid[:])
            wr_sb = constp.tile([128, KD, E], f32)
            nc.sync.dma_start(out=wr_sb[:], in_=wr[:])
            b1_sb = constp.tile([128, E, MF], f32)
            nc.sync.dma_start(out=b1_sb[:], in_=b1[:])
            b2_sb = constp.tile([1, E, D], bf16)
            nc.sync.dma_start(out=b2_sb[:], in_=b2[None, :, :])

            # default-fill dispatch tables
            nc.sync.dma_start(out=tokdram[:], in_=tokfill[:])
            zero_sb = constp.tile([128, D], f32)
            nc.vector.memset(zero_sb[:], 0.0)
            # zero the output accumulator and the cw table
            for c in range(NT):
                nc.sync.dma_start(out=out[128 * c : 128 * (c + 1), :], in_=zero_sb[:])
            nc.sync.dma_start(out=out[T : T + 1, :], in_=zero_sb[0:1, :])
            nc.sync.dma_start(out=cwdram[:], in_=zero_sb[:, : NSLOT // 128])

            # dispatch tables that outlive the router scope
            idxs_sb = routr.tile([128, NSLOT // 16], i16)
            cw_sb = routr.tile([128, NSLOT // 128], f32)

            router_phase(
                nc, tc, xT, tokdram, cwdram, aux,
                lt_sb, ones_sb, ecap_sb, tokid_sb, wr_sb, idxs_sb, cw_sb,
            )
            ffn_phase(
                nc, tc, xbf, w1, w2, out,
                b1_sb, b2_sb, onesbf_sb, idxs_sb, cw_sb,
            )

    nc.compile()
    return nc


_NC_CACHE = None


def _get_nc():
    global _NC_CACHE
    if _NC_CACHE is None:
        _NC_CACHE = build_program()
    return _NC_CACHE


def make_in_maps(x, Wr, W1, b1, W2, b2):
    xf = np.ascontiguousarray(np.asarray(x, np.float32).reshape(B * S, D))
    W1bf = np.asarray(W1, np.float32).astype(BF16)
    W2bf = np.asarray(W2, np.float32).astype(BF16)
    b2bf = np.asarray(b2, np.float32).astype(BF16)
    wr_p = np.ascontiguousarray(
        np.asarray(Wr, np.float32).reshape(KD, 128, E).transpose(1, 0, 2)
    )
    b1_p = np.ascontiguousarray(
        np.asarray(b1, np.float32).reshape(E, MF, 128).transpose(2, 0, 1)
    )
    lt_c = np.triu(np.ones((128, 128), np.float32))
    ecap_c = np.broadcast_to((np.arange(E) * C).astype(np.float32), (128, E)).copy()
    tokid_c = (
        (np.arange(2 * NT)[None, :] % NT) * 128 + np.arange(128)[:, None]
    ).astype(np.int16)
    tokfill_c = np.full((16, NSLOT // 16), DUMP, np.int16)
    ones_c = np.ones((128, 128), np.float32)
    onesbf_c = np.ones((1, 128), BF16)

    shared = dict(
        wr=wr_p, w1=W1bf, w2=W2bf, b1=b1_p, b2=b2bf, lt=lt_c, ecap=ecap_c,
        tokid=tokid_c, tokfill=tokfill_c, ones=ones_c, onesbf=onesbf_c,
    )
    in_maps = []
    for c in range(NCORES):
        xs = xf[T * c : T * (c + 1)]
        xbf_c = np.zeros((T + 1, D), BF16)
        xbf_c[:T] = xs.astype(BF16)
        m = dict(shared)
        m["xT"] = np.ascontiguousarray(xs.T)
        m["xbf"] = xbf_c
        in_maps.append(m)
    return in_maps


def finish(results):
    outp = np.empty((B * S, D), np.float32)
    usage_sum = np.zeros(E, np.float64)
    zsq_sum = 0.0
    for c in range(NCORES):
        outp[T * c : T * (c + 1)] = results[c]["out"][:T]
        a = results[c]["aux"][0]
        usage_sum += a[:E].astype(np.float64)
        zsq_sum += float(a[E])
    n = float(B * S)
    usage = (usage_sum / n).astype(np.float32)
    mean_u = np.float32(usage.mean())
    var_u = np.float32(np.mean((usage - mean_u) ** 2))
    lb = var_u / (mean_u * mean_u + np.float32(1e-8)) * np.float32(E) * np.float32(0.01)
    z = np.float32(zsq_sum / n) * np.float32(0.001)
    return outp.reshape(B, S, D), np.float32(lb), np.float32(z)


def kernel(x, Wr, W1, b1, W2, b2):
    nc = _get_nc()
    in_maps = make_in_maps(x, Wr, W1, b1, W2, b2)
    res = run_bass_kernel_spmd(nc, in_maps, list(range(NCORES)))
    return finish(res.results)


# revision 23
# speedup vs baseline: 1.2979x; 1.2247x over previous
"""MoE (B=4,S=2048,D=768,F=3072,E=8,K=2) on 8 TRN2 NeuronCores.

Strategy: data-parallel over tokens (1024 tokens/core), sparse top-2 expert
compute with capacity-padded per-expert buckets (C=384 slots/expert/core).
Router, top-2 selection, dispatch-index build, FFN, and weighted combine all
run on-device; the host only shards inputs, sums 8 tiny per-core partial
vectors for the aux losses, and reassembles the output.
"""

import numpy as np
import ml_dtypes

import concourse.bass as bass
import concourse.mybir as mybir
import concourse.tile as tile
from concourse import bacc
from concourse.bass_utils import run_bass_kernel_spmd

BF16 = ml_dtypes.bfloat16

B, S, D, F, E, K = 4, 2048, 768, 3072, 8, 2
NCORES = 8
T = (B * S) // NCORES          # 1024 tokens per core
NT = T // 128                  # 8 token tiles
C = 384                        # per-expert capacity (multiple of 128)
NSLOT = E * C                  # 3072 dispatch slots
DUMP = T                       # dump row index (row T of xbf / out)
KD = D // 128                  # 6 k-tiles over D
MF = F // 128                  # 24 m-tiles over F
CT = C // 128                  # 3 slot tiles per expert
BIGOFF = 16384.0               # scatter-skip offset (> bounds_check, f32-exact range)

DEBUG = False

f32 = mybir.dt.float32
bf16 = mybir.dt.bfloat16
i16 = mybir.dt.int16
i32 = mybir.dt.int32


def router_phase(nc, tc, xT, tokdram, cwdram, aux,
                 lt_sb, ones_sb, ecap_sb, tokid_sb, wr_sb, idxs_sb, cw_sb,
                 routr_pool):
    """Batched router: all NT token tiles processed as [128, NT, E] tensors.
    Returns (cw_scatter_emit) callback to be invoked after the gathers are
    emitted, so the cw-table scatters queue behind them on the Q7 FIFO."""
    with (
        tc.tile_pool(name="rpsum", bufs=2, space="PSUM") as rpsum,
        tc.tile_pool(name="rpsum1", bufs=1, space="PSUM") as rpsum1,
        tc.tile_pool(name="rtmp", bufs=2) as rtmp,
    ):
        xT_sb = rtmp.tile([128, KD, T], f32, tag="xT")
        nc.sync.dma_start(out=xT_sb[:], in_=xT[:, :].rearrange("(k p) t -> p k t", p=128))

        l_all = rtmp.tile([128, NT, E], f32, tag="l_all")
        for c in range(NT):
            lg_ps = rpsum.tile([128, E], f32, tag="lg")
            for k in range(KD):
                nc.tensor.matmul(
                    lg_ps[:], lhsT=xT_sb[:, k, 128 * c : 128 * (c + 1)],
                    rhs=wr_sb[:, k, :],
                    start=(k == 0), stop=(k == KD - 1),
                )
            nc.vector.tensor_copy(l_all[:, c, :], lg_ps[:])

        def bc(t2):  # [128, NT] -> [128, NT, E] broadcast
            return t2[:, :, None].to_broadcast([128, NT, E])

        m1 = rtmp.tile([128, NT], f32, tag="m1")
        nc.vector.reduce_max(m1[:], l_all[:], axis=mybir.AxisListType.X)
        is1 = rtmp.tile([128, NT, E], f32, tag="is1")
        nc.vector.tensor_tensor(is1[:], l_all[:], bc(m1), mybir.AluOpType.is_ge)
        lmask = rtmp.tile([128, NT, E], f32, tag="lmask")
        nc.vector.scalar_tensor_tensor(
            out=lmask[:], in0=is1[:], scalar=-1e9, in1=l_all[:],
            op0=mybir.AluOpType.mult, op1=mybir.AluOpType.add,
        )
        m2 = rtmp.tile([128, NT], f32, tag="m2")
        nc.vector.reduce_max(m2[:], lmask[:], axis=mybir.AxisListType.X)

        lsub = rtmp.tile([128, NT, E], f32, tag="lsub")
        nc.vector.tensor_tensor(lsub[:], l_all[:], bc(m1), mybir.AluOpType.subtract)
        ex = rtmp.tile([128, NT, E], f32, tag="ex")
        nc.scalar.activation(ex[:], lsub[:], mybir.ActivationFunctionType.Exp)
        sume = rtmp.tile([128, NT], f32, tag="sume")
        nc.vector.reduce_sum(sume[:], ex[:], axis=mybir.AxisListType.X)
        rcps = rtmp.tile([128, NT], f32, tag="rcps")
        nc.vector.reciprocal(rcps[:], sume[:])
        p = rtmp.tile([128, NT, E], f32, tag="p")
        nc.vector.tensor_tensor(p[:], ex[:], bc(rcps), mybir.AluOpType.mult)

        # aux partials
        usage_acc = rtmp.tile([128, E], f32, tag="usage")
        nc.vector.reduce_sum(
            usage_acc[:], p[:, :, :].rearrange("p c e -> p e c"),
            axis=mybir.AxisListType.X,
        )
        lns = rtmp.tile([128, NT], f32, tag="lns")
        nc.scalar.activation(lns[:], sume[:], mybir.ActivationFunctionType.Ln)
        z = rtmp.tile([128, NT], f32, tag="z")
        nc.vector.tensor_add(z[:], lns[:], m1[:])
        zsq_acc = rtmp.tile([128, 1], f32, tag="zsq")
        z2 = rtmp.tile([128, NT], f32, tag="z2")
        nc.vector.tensor_mul(z2[:], z[:], z[:])
        nc.vector.reduce_sum(zsq_acc[:], z2[:], axis=mybir.AxisListType.X)

        mask2 = rtmp.tile([128, NT, E], f32, tag="mask2")
        nc.vector.tensor_tensor(mask2[:], l_all[:], bc(m2), mybir.AluOpType.is_ge)
        psel = rtmp.tile([128, NT, E], f32, tag="psel")
        nc.vector.tensor_mul(psel[:], p[:], mask2[:])
        s12 = rtmp.tile([128, NT], f32, tag="s12")
        nc.vector.reduce_sum(s12[:], psel[:], axis=mybir.AxisListType.X)
        nc.vector.tensor_scalar_add(s12[:], s12[:], 1e-8)
        rcpd = rtmp.tile([128, NT], f32, tag="rcpd")
        nc.vector.reciprocal(rcpd[:], s12[:])
        cw = rtmp.tile([128, NT, E], f32, tag="cw")
        nc.vector.tensor_tensor(cw[:], psel[:], bc(rcpd), mybir.AluOpType.mult)

        # per-tile inclusive cumsum (no carry), evicted as pos = cum - mask
        pos = rtmp.tile([128, NT, E], f32, tag="pos")
        for c in range(NT):
            cum_ps = rpsum.tile([128, E], f32, tag="cum")
            nc.tensor.matmul(cum_ps[:], lhsT=lt_sb[:], rhs=mask2[:, c, :],
                             start=True, stop=True)
            nc.vector.tensor_sub(pos[:, c, :], cum_ps[:], mask2[:, c, :])
        # tile totals in one matmul, then exclusive prefix over tiles on [1, E]
        tot_ps = rpsum1.tile([1, NT, E], f32, tag="tot")
        nc.tensor.matmul(
            tot_ps[:], lhsT=ones_sb[:, 0:1],
            rhs=mask2[:, :, :].rearrange("p c e -> p (c e)"),
            start=True, stop=True,
        )
        carry = rtmp.tile([1, NT, E], f32, tag="carry")
        nc.vector.memset(carry[0:1, 0, :], 0.0)
        for c in range(1, NT):
            nc.vector.tensor_add(carry[0:1, c, :], carry[0:1, c - 1, :],
                                 tot_ps[0:1, c - 1, :])
        carry_bc = rpsum1.tile([128, NT, E], f32, tag="carrybc")
        nc.tensor.matmul(
            carry_bc[:], lhsT=ones_sb[0:1, :],
            rhs=carry[:, :, :].rearrange("p c e -> p (c e)"),
            start=True, stop=True,
        )
        nc.vector.tensor_add(pos[:], pos[:], carry_bc[:])

        s_all = rtmp.tile([128, NT, E], f32, tag="s_all")
        nc.vector.tensor_tensor(
            s_all[:], pos[:], ecap_sb[:, None, :].to_broadcast([128, NT, E]),
            mybir.AluOpType.add,
        )
        ovf = rtmp.tile([128, NT, E], f32, tag="ovf")
        nc.vector.tensor_scalar(ovf[:], pos[:], float(C), None, mybir.AluOpType.is_ge)
        valid = rtmp.tile([128, NT, E], f32, tag="valid")
        nc.vector.scalar_tensor_tensor(
            out=valid[:], in0=ovf[:], scalar=-1.0, in1=mask2[:],
            op0=mybir.AluOpType.add, op1=mybir.AluOpType.mult,
        )
        nc.vector.tensor_scalar_mul(valid[:], valid[:], -1.0)
        cweff = rtmp.tile([128, NT, E], f32, tag="cweff")
        nc.vector.tensor_mul(cweff[:], cw[:], valid[:])
        is2 = rtmp.tile([128, NT, E], f32, tag="is2")
        nc.vector.tensor_sub(is2[:], mask2[:], is1[:])

        if DEBUG:
            for c in range(NT):
                nc.sync.dma_start(out=nc._dbg[0][128 * c : 128 * (c + 1), 0:E], in_=mask2[:, c, :])
                nc.sync.dma_start(out=nc._dbg[0][128 * c : 128 * (c + 1), E : 2 * E], in_=pos[:, c, :])
                nc.sync.dma_start(out=nc._dbg[0][128 * c : 128 * (c + 1), 2 * E : 3 * E], in_=is1[:, c, :])
                nc.sync.dma_start(out=nc._dbg[0][128 * c : 128 * (c + 1), 3 * E : 4 * E], in_=l_all[:, c, :])
                nc.sync.dma_start(out=nc._dbg[0][128 * c : 128 * (c + 1), 4 * E : 5 * E], in_=s_all[:, c, :])

        # batched extraction per k slot
        sk = {}; vk = {}; cwk = {}
        for ki, sel in ((0, is1), (1, is2)):
            tmp_a = rtmp.tile([128, NT, E], f32, tag=f"xa{ki}")
            nc.vector.tensor_mul(tmp_a[:], s_all[:], sel[:])
            sk[ki] = rtmp.tile([128, NT], f32, tag=f"sk{ki}", name=f"sk{ki}")
            nc.vector.reduce_sum(sk[ki][:], tmp_a[:], axis=mybir.AxisListType.X)
            tmp_b = rtmp.tile([128, NT, E], f32, tag=f"xb{ki}")
            nc.vector.tensor_mul(tmp_b[:], valid[:], sel[:])
            vk[ki] = rtmp.tile([128, NT], f32, tag=f"vk{ki}", name=f"vk{ki}")
            nc.vector.reduce_sum(vk[ki][:], tmp_b[:], axis=mybir.AxisListType.X)
            tmp_c = rtmp.tile([128, NT, E], f32, tag=f"xc{ki}")
            nc.vector.tensor_mul(tmp_c[:], cweff[:], sel[:])
            cwk[ki] = routr_pool.tile([128, NT], f32, tag=f"cwk{ki}", name=f"cwk{ki}")
            nc.vector.reduce_sum(cwk[ki][:], tmp_c[:], axis=mybir.AxisListType.X)

        # batched wrapped-layout offset transforms
        oi = {}
        for table, shift, ncol in (("tok", 4, float(NSLOT // 16)),
                                   ("cw", 7, float(NSLOT // 128))):
            for ki in range(2):
                q_skip = float(NSLOT * (1 << shift))
                t1 = rtmp.tile([128, NT], f32, tag=f"t1{table}{ki}")
                nc.vector.tensor_scalar(
                    t1[:], vk[ki][:], -q_skip, q_skip,
                    mybir.AluOpType.mult, mybir.AluOpType.add,
                )
                sm = rtmp.tile([128, NT], f32, tag=f"sm{table}{ki}")
                nc.vector.tensor_add(sm[:], sk[ki][:], t1[:])
                smi = rtmp.tile([128, NT], i32, tag=f"smi{table}{ki}")
                nc.vector.tensor_copy(smi[:], sm[:])
                qi = rtmp.tile([128, NT], i32, tag=f"qi{table}{ki}")
                nc.vector.tensor_scalar(
                    qi[:], smi[:], shift, None, mybir.AluOpType.arith_shift_right
                )
                qf = rtmp.tile([128, NT], f32, tag=f"qf{table}{ki}")
                nc.vector.tensor_copy(qf[:], qi[:])
                af = rtmp.tile([128, NT], f32, tag=f"af{table}{ki}")
                nc.vector.tensor_scalar_mul(af[:], sm[:], ncol)
                of = rtmp.tile([128, NT], f32, tag=f"of{table}{ki}")
                nc.vector.scalar_tensor_tensor(
                    out=of[:], in0=qf[:], scalar=-float(NSLOT - 1), in1=af[:],
                    op0=mybir.AluOpType.mult, op1=mybir.AluOpType.add,
                )
                o = routr_pool.tile([128, NT], i32, tag=f"oi{table}{ki}", name=f"oi{table}{ki}")
                nc.vector.tensor_copy(o[:], of[:])
                oi[(table, ki)] = o
                if DEBUG:
                    dst = nc._dbg[1][:, 2 * NT : 4 * NT] if table == "zz" else None
                if DEBUG and table == "tok":
                    nc.sync.dma_start(out=nc._dbg[1][:, NT * ki : NT * (ki + 1)], in_=of[:])
                if DEBUG and table == "cw":
                    nc.sync.dma_start(out=nc._dbg[1][:, 4 * NT + NT * ki : 4 * NT + NT * (ki + 1)], in_=of[:])
        if DEBUG:
            nc.sync.dma_start(out=nc._dbg[1][:, 2 * NT : 3 * NT], in_=cwk[0][:])
            nc.sync.dma_start(out=nc._dbg[1][:, 3 * NT : 4 * NT], in_=cwk[1][:])

        # tok-table scatters (gate the gathers -> emit first on the Q7 FIFO)
        for ki in range(2):
            for c in range(NT):
                nc.gpsimd.indirect_dma_start(
                    out=tokdram[:, :].rearrange("a b -> (a b)")[:, None],
                    out_offset=bass.IndirectOffsetOnAxis(
                        ap=oi[("tok", ki)][:, c : c + 1], axis=0),
                    in_=tokid_sb[:, NT * ki + c : NT * ki + c + 1],
                    in_offset=None,
                    bounds_check=NSLOT - 1,
                    oob_is_err=False,
                )

        # aux partial sums -> [1,16]
        aux_sb = rtmp.tile([1, 16], f32, tag="auxsb")
        nc.vector.memset(aux_sb[:], 0.0)
        us_ps = rpsum1.tile([1, E], f32, tag="usps")
        nc.tensor.matmul(us_ps[:], lhsT=ones_sb[:, 0:1], rhs=usage_acc[:], start=True, stop=True)
        nc.vector.tensor_copy(aux_sb[0:1, 0:E], us_ps[:])
        zs_ps = rpsum1.tile([1, 1], f32, tag="zsps")
        nc.tensor.matmul(zs_ps[:], lhsT=ones_sb[:, 0:1], rhs=zsq_acc[:], start=True, stop=True)
        nc.vector.tensor_copy(aux_sb[0:1, E : E + 1], zs_ps[:])
        nc.sync.dma_start(out=aux[:], in_=aux_sb[:])

        # load the tok table back (wrapped idx layout, replicated x8)
        for g in range(8):
            nc.sync.dma_start(out=idxs_sb[16 * g : 16 * (g + 1), :], in_=tokdram[:])

    def emit_cw_scatters():
        for ki in range(2):
            for c in range(NT):
                nc.gpsimd.indirect_dma_start(
                    out=cwdram[:, :].rearrange("a b -> (a b)")[:, None],
                    out_offset=bass.IndirectOffsetOnAxis(
                        ap=oi[("cw", ki)][:, c : c + 1], axis=0),
                    in_=cwk[ki][:, c : c + 1],
                    in_offset=None,
                    bounds_check=NSLOT - 1,
                    oob_is_err=False,
                )
        nc.sync.dma_start(out=cw_sb[:], in_=cwdram[:])

    return emit_cw_scatters


def ffn_phase(nc, tc, xbf, w1, w2, out, b1_sb, b2_sb, onesbf_sb, idxs_sb, cw_sb, emit_cw_scatters):
    with (
        tc.tile_pool(name="w1p", bufs=2) as w1p,
        tc.tile_pool(name="w2p", bufs=8) as w2p,
        tc.tile_pool(name="xtp", bufs=1) as xtp,
        tc.tile_pool(name="hp", bufs=2) as hp,
        tc.tile_pool(name="yp", bufs=3) as yp,
        tc.tile_pool(name="ps1", bufs=2, space="PSUM") as ps1p,
        tc.tile_pool(name="ps2", bufs=1, space="PSUM") as ps2p,
    ):
        xt_all = []
        for e in range(E):
            xt_e = xtp.tile([128, KD, C], bf16, tag=f"xt{e}", name=f"xt{e}")
            nc.gpsimd.dma_gather(
                out_ap=xt_e[:],
                in_ap=xbf[:, :],
                idxs_ap=idxs_sb[:, (C // 16) * e : (C // 16) * (e + 1)],
                num_idxs=C,
                num_idxs_reg=C,
                elem_size=D,
                transpose=True,
            )
            xt_all.append(xt_e)
        emit_cw_scatters()
        for e in range(E):
            xt_e = xt_all[e]
            if DEBUG:
                nc.sync.dma_start(out=nc._dbg[2][:, e, :, :], in_=xt_e[:])
            w1_e = w1p.tile([128, KD, F], bf16, tag="w1")
            nc.sync.dma_start(
                out=w1_e[:], in_=w1[e, :, :].rearrange("(k p) f -> p k f", p=128)
            )
            h_e = hp.tile([128, MF, C], bf16, tag="h")
            for m in range(MF):
                ps = ps1p.tile([128, C], f32, tag="ps1")
                for k in range(KD):
                    nc.tensor.matmul(
                        ps[:],
                        lhsT=w1_e[:, k, 128 * m : 128 * (m + 1)],
                        rhs=xt_e[:, k, :],
                        start=(k == 0),
                        stop=(k == KD - 1),
                    )
                nc.scalar.activation(
                    h_e[:, m, :], ps[:], mybir.ActivationFunctionType.Gelu,
                    bias=b1_sb[:, e, m : m + 1], scale=1.0,
                )
            if DEBUG:
                nc.sync.dma_start(out=nc._dbg[3][:, e, :, :], in_=h_e[:, 0:4, :])

            # mm2: k-outer over F, 6 live PSUM banks (3 slot-tiles x 2 halves)
            ps2 = [
                [
                    ps2p.tile(
                        [128, D // 2], f32,
                        tag=f"ps2_{mt}_{nh}", name=f"ps2_{mt}_{nh}",
                    )
                    for nh in range(2)
                ]
                for mt in range(CT)
            ]
            for k in range(MF):
                w2_k = w2p.tile([128, D], bf16, tag="w2")
                nc.sync.dma_start(out=w2_k[:], in_=w2[e, 128 * k : 128 * (k + 1), :])
                for mt in range(CT):
                    for nh in range(2):
                        nc.tensor.matmul(
                            ps2[mt][nh][:],
                            lhsT=h_e[:, k, 128 * mt : 128 * (mt + 1)],
                            rhs=w2_k[:, (D // 2) * nh : (D // 2) * (nh + 1)],
                            start=(k == 0),
                            stop=False,
                        )
            for mt in range(CT):
                for nh in range(2):
                    nc.tensor.matmul(
                        ps2[mt][nh][:],
                        lhsT=onesbf_sb[:],
                        rhs=b2_sb[0:1, e, (D // 2) * nh : (D // 2) * (nh + 1)],
                        start=False,
                        stop=True,
                    )
            for mt in range(CT):
                y_sb = yp.tile([128, 1, D], f32, tag="y")
                slot_col = CT * e + mt
                for nh in range(2):
                    nc.vector.tensor_scalar_mul(
                        y_sb[:, 0, (D // 2) * nh : (D // 2) * (nh + 1)],
                        ps2[mt][nh][:],
                        cw_sb[:, slot_col : slot_col + 1],
                    )
                nc.gpsimd.dma_scatter_add(
                    out_ap=out[:, :],
                    in_ap=y_sb[:],
                    idxs_ap=idxs_sb[:, (C // 16) * e + 8 * mt : (C // 16) * e + 8 * (mt + 1)],
                    num_idxs=128,
                    num_idxs_reg=128,
                    elem_size=D,
                )


def build_program():
    nc = bacc.Bacc(None, target_bir_lowering=False)

    # ---- DRAM I/O ----
    xT = nc.declare_dram_parameter("xT", [D, T], f32, isOutput=False)
    xbf = nc.declare_dram_parameter("xbf", [T + 1, D], bf16, isOutput=False)
    wr = nc.declare_dram_parameter("wr", [128, KD, E], f32, isOutput=False)
    w1 = nc.declare_dram_parameter("w1", [E, D, F], bf16, isOutput=False)
    w2 = nc.declare_dram_parameter("w2", [E, F, D], bf16, isOutput=False)
    b1 = nc.declare_dram_parameter("b1", [128, E, MF], f32, isOutput=False)
    b2 = nc.declare_dram_parameter("b2", [E, D], bf16, isOutput=False)
    lt = nc.declare_dram_parameter("lt", [128, 128], f32, isOutput=False)
    ecap = nc.declare_dram_parameter("ecap", [128, E], f32, isOutput=False)
    tokid = nc.declare_dram_parameter("tokid", [128, 2 * NT], i16, isOutput=False)
    tokfill = nc.declare_dram_parameter("tokfill", [16, NSLOT // 16], i16, isOutput=False)
    ones = nc.declare_dram_parameter("ones", [128, 128], f32, isOutput=False)
    onesbf = nc.declare_dram_parameter("onesbf", [1, 128], bf16, isOutput=False)

    out = nc.declare_dram_parameter("out", [T + 1, D], f32, isOutput=True)
    aux = nc.declare_dram_parameter("aux", [1, 16], f32, isOutput=True)

    if DEBUG:
        nc._dbg = [
            nc.declare_dram_parameter("dbg", [T, 6 * E], f32, isOutput=True),
            nc.declare_dram_parameter("dbg2", [128, 6 * NT], f32, isOutput=True),
            nc.declare_dram_parameter("dbgxt", [128, E, KD, C], bf16, isOutput=True),
            nc.declare_dram_parameter("dbgh", [128, E, 4, C], bf16, isOutput=True),
            nc.declare_dram_parameter("dbgidx", [128, NSLOT // 16], f32, isOutput=True),
            nc.declare_dram_parameter("dbgcw", [128, NSLOT // 128], f32, isOutput=True),
        ]

    # ---- DRAM internal ----
    tokdram = nc.dram_tensor("tokdram", [16, NSLOT // 16], i16)
    cwdram = nc.dram_tensor("cwdram", [128, NSLOT // 128], f32)

    with tile.TileContext(nc) as tc:
        with (
            tc.tile_pool(name="const", bufs=1) as constp,
            tc.tile_pool(name="routr", bufs=1) as routr,
        ):
            # ---------- phase 0: constants + init ----------
            lt_sb = constp.tile([128, 128], f32)
            nc.sync.dma_start(out=lt_sb[:], in_=lt[:])
            ones_sb = constp.tile([128, 128], f32)
            nc.sync.dma_start(out=ones_sb[:], in_=ones[:])
            onesbf_sb = constp.tile([1, 128], bf16)
            nc.sync.dma_start(out=onesbf_sb[:], in_=onesbf[:])
            ecap_sb = constp.tile([128, E], f32)
            nc.sync.dma_start(out=ecap_sb[:], in_=ecap[:])
            tokid_sb = constp.tile([128, 2 * NT], i16)
            nc.sync.dma_start(out=tokid_sb[:], in_=tokid[:])
            wr_sb = constp.tile([128, KD, E], f32)
            nc.sync.dma_start(out=wr_sb[:], in_=wr[:])
            b1_sb = constp.tile([128, E, MF], f32)
            nc.sync.dma_start(out=b1_sb[:], in_=b1[:])
            b2_sb = constp.tile([1, E, D], bf16)
            nc.sync.dma_start(out=b2_sb[:], in_=b2[None, :, :])

            # default-fill dispatch tables
            nc.sync.dma_start(out=tokdram[:], in_=tokfill[:])
            zero_sb = constp.tile([128, D], f32)
            nc.vector.memset(zero_sb[:], 0.0)
            # zero the output accumulator and the cw table
            for c in range(NT):
                nc.sync.dma_start(out=out[128 * c : 128 * (c + 1), :], in_=zero_sb[:])
            nc.sync.dma_start(out=out[T : T + 1, :], in_=zero_sb[0:1, :])
            nc.sync.dma_start(out=cwdram[:], in_=zero_sb[:, : NSLOT // 128])

            # dispatch tables that outlive the router scope
            idxs_sb = routr.tile([128, NSLOT // 16], i16)
            cw_sb = routr.tile([128, NSLOT // 128], f32)

            emit_cw = router_phase(
                nc, tc, xT, tokdram, cwdram, aux,
                lt_sb, ones_sb, ecap_sb, tokid_sb, wr_sb, idxs_sb, cw_sb,
                routr,
            )
            ffn_phase(
                nc, tc, xbf, w1, w2, out,
                b1_sb, b2_sb, onesbf_sb, idxs_sb, cw_sb, emit_cw,
            )

    nc.compile()
    return nc


_NC_CACHE = None


def _get_nc():
    global _NC_CACHE
    if _NC_CACHE is None:
        _NC_CACHE = build_program()
    return _NC_CACHE


def make_in_maps(x, Wr, W1, b1, W2, b2):
    xf = np.ascontiguousarray(np.asarray(x, np.float32).reshape(B * S, D))
    W1bf = np.asarray(W1, np.float32).astype(BF16)
    W2bf = np.asarray(W2, np.float32).astype(BF16)
    b2bf = np.asarray(b2, np.float32).astype(BF16)
    wr_p = np.ascontiguousarray(
        np.asarray(Wr, np.float32).reshape(KD, 128, E).transpose(1, 0, 2)
    )
    b1_p = np.ascontiguousarray(
        np.asarray(b1, np.float32).reshape(E, MF, 128).transpose(2, 0, 1)
    )
    lt_c = np.triu(np.ones((128, 128), np.float32))
    ecap_c = np.broadcast_to((np.arange(E) * C).astype(np.float32), (128, E)).copy()
    tokid_c = (
        (np.arange(2 * NT)[None, :] % NT) * 128 + np.arange(128)[:, None]
    ).astype(np.int16)
    tokfill_c = np.full((16, NSLOT // 16), DUMP, np.int16)
    ones_c = np.ones((128, 128), np.float32)
    onesbf_c = np.ones((1, 128), BF16)

    shared = dict(
        wr=wr_p, w1=W1bf, w2=W2bf, b1=b1_p, b2=b2bf, lt=lt_c, ecap=ecap_c,
        tokid=tokid_c, tokfill=tokfill_c, ones=ones_c, onesbf=onesbf_c,
    )
    in_maps = []
    for c in range(NCORES):
        xs = xf[T * c : T * (c + 1)]
        xbf_c = np.zeros((T + 1, D), BF16)
        xbf_c[:T] = xs.astype(BF16)
        m = dict(shared)
        m["xT"] = np.ascontiguousarray(xs.T)
        m["xbf"] = xbf_c
        in_maps.append(m)
    return in_maps


def finish(results):
    outp = np.empty((B * S, D), np.float32)
    usage_sum = np.zeros(E, np.float64)
    zsq_sum = 0.0
    for c in range(NCORES):
        outp[T * c : T * (c + 1)] = results[c]["out"][:T]
        a = results[c]["aux"][0]
        usage_sum += a[:E].astype(np.float64)
        zsq_sum += float(a[E])
    n = float(B * S)
    usage = (usage_sum / n).astype(np.float32)
    mean_u = np.float32(usage.mean())
    var_u = np.float32(np.mean((usage - mean_u) ** 2))
    lb = var_u / (mean_u * mean_u + np.float32(1e-8)) * np.float32(E) * np.float32(0.01)
    z = np.float32(zsq_sum / n) * np.float32(0.001)
    return outp.reshape(B, S, D), np.float32(lb), np.float32(z)


def kernel(x, Wr, W1, b1, W2, b2):
    nc = _get_nc()
    in_maps = make_in_maps(x, Wr, W1, b1, W2, b2)
    res = run_bass_kernel_spmd(nc, in_maps, list(range(NCORES)))
    return finish(res.results)
